# revision 9
# baseline (speedup 1.0000x reference)
"""Trainium2 Bass kernel for CombinedRepeatCausalLinear (parallel forward).

Computes out[b,e,t] = sum_s x[b,e,s] * W[s,t] + bias[t] where
  W[s,t] = mask(t>=s) * (w0[s]*d0^(t-s) + w1[t]*d1^(t-s))
for S = 2048, x of shape (8, 1024, 2048) fp32.

W is a causally-masked rank-2 matrix, so x @ W is a 2-state linear
recurrence along t, evaluated as a chunked scan: 17 diagonal chunks of
L=126 columns. Per chunk and 512-wide r-block:
  MM_a (K=128): causal-decay x-block matmul -> psum (start, no stop)
  MM_b (K=2):   carry-in contribution, accumulated (stop). Its moving
                operand is the PREVIOUS chunk's drained output tile
                rows 96..97 -- the drain op itself transports the
                carry (A,B) values to SBUF, so no separate carry copy
                exists anywhere.
  drain:        psum -> fp16 out tile + bias; r-block 0 on DVE
                (tensor_scalar add), r-block 1 on ScalarE (activation).
The serial chain link is drain -> K=2 matmul (~1us); the two r-block
lanes interleave on the PE.

Layout/perf notes:
- Compute-engine SBUF accesses must start at a 32-aligned partition,
  so carry rows/cols sit at partitions 96..97 and x/out rows occupy
  [0..95, 98..127] (stationary host-permuted to match). The psum
  carry cols land at 96..97 with zero bias, and MM_b uses
  tile_position=(96,0) to contract over PE rows 96..97.
- fp16 end-to-end I/O: 4MB x in + 4MB out + 0.5MB W per core.
- x and out are PACKED 2 chunks per DMA instruction ([128, 2048] fp16
  tiles = 4KB contiguous per partition; per-queue DMA throughput is
  packet-size-bound), alternating between the two HWDGE queues
  (sync + scalar).
- Each chunk uses a 2-bank PSUM tile [128,1024]; the two r-block
  matmul groups write bank-aligned halves.
- 10 small dummy matmuls run during the initial DMA wait to warm the
  PE clock (HAM un-throttle) before the chain starts.
"""

import numpy as np

import concourse.bass as bass
import concourse.mybir as mybir
import concourse.tile as tile
from concourse import bacc
from concourse.bass_utils import run_bass_kernel_spmd

F32 = mybir.dt.float32
F16 = mybir.dt.float16

B = 8
E = 1024
S = 2048
DC = 1.0
N_CORES = 8
R = (B * E) // N_CORES      # rows per core = 1024
L = 126                     # chunk length (+2 carry rows = 128 partitions)
RB = 2                      # r-blocks of 512 (fp32-psum bank width)
PACKW = 2                   # chunks per packed DMA
NPACK = 8                   # packs of full chunks (16 full chunks)
NFULL = NPACK * PACKW       # 16
SLAST = NFULL * L           # 2016
LLAST = S - SLAST           # 32
NCH = NFULL + 1             # 17
NWB = NCH + 1               # wpk column blocks (18th = last-chunk carry)
NDUMMY = 10                 # PE warm-up matmuls

# partition p -> s_rel within a full chunk (96..97 are carry slots)
SREL = list(range(96)) + [None, None] + list(range(96, L))
# logical row/col permutation applied to the [128,128] stationary block
PERM = list(range(96)) + [126, 127] + list(range(96, 126))

_PROGRAM = None


def _build_program():
    nc = bacc.Bacc("TRN2", target_bir_lowering=False, debug=False,
                   num_devices=N_CORES)

    xpk_d = nc.declare_dram_parameter("xpk", [128, NPACK, PACKW * R], F16,
                                      isOutput=False)
    xlast_d = nc.declare_dram_parameter("xlast", [LLAST, R], F16,
                                        isOutput=False)
    wpk_d = nc.declare_dram_parameter("wpk", [128, NWB * 128], F16,
                                      isOutput=False)
    biasT_d = nc.declare_dram_parameter("biasT", [128, NCH], F32,
                                        isOutput=False)
    opk_d = nc.declare_dram_parameter("opk", [128, NPACK, PACKW * R], F16,
                                      isOutput=True)
    olast_d = nc.declare_dram_parameter("olast", [LLAST, R], F16,
                                        isOutput=True)

    with tile.TileContext(nc) as tc:
        with (
            tc.tile_pool(name="cst", bufs=1) as cst,
            tc.tile_pool(name="xp", bufs=1) as xp,
            tc.tile_pool(name="op", bufs=1) as op,
            tc.tile_pool(name="dum", bufs=1) as dum,
            tc.tile_pool(name="ps", bufs=3, space="PSUM") as psp,
            tc.tile_pool(name="pd", bufs=2, space="PSUM") as pdp,
        ):
            # ---- PE warm-up: dummy matmuls on memset tiles ----
            wdum = dum.tile([128, 128], F16, tag="wdum")
            xdum = dum.tile([128, 128], F16, tag="xdum")
            nc.vector.memset(wdum[:], 0.0)
            nc.vector.memset(xdum[:], 0.0)
            for i in range(NDUMMY):
                pd = pdp.tile([128, 128], F32, tag="pd", name=f"pd{i}")
                nc.tensor.matmul(pd[:], wdum[:], xdum[:],
                                 start=True, stop=True)

            # ---- constants: packed W + bias (scalar queue, first) ----
            wpk = cst.tile([128, NWB * 128], F16, tag="wpk")
            nc.scalar.dma_start(wpk[:], wpk_d[:])
            bias_sb = cst.tile([128, NCH], F32, tag="bias")
            nc.scalar.dma_start(bias_sb[:], biasT_d[:])

            # ---- x input: packs alternating across the two HWDGE queues ----
            xt = []
            for q in range(NPACK):
                t = xp.tile([128, PACKW * R], F16, tag=f"xq{q}",
                            name=f"xq{q}")
                eng = nc.sync if q % 2 == 0 else nc.scalar
                eng.dma_start(t[:], xpk_d[:, q, :])
                xt.append(t)
            xlast = xp.tile([LLAST, R], F16, tag="xlast")
            nc.sync.dma_start(xlast[:], xlast_d[:])

            ot = [op.tile([128, PACKW * R], F16, tag=f"oq{q}", name=f"oq{q}")
                  for q in range(NPACK)]
            olast = op.tile([128, R], F16, tag="olast")

            def chunk_geom(c):
                if c == NCH - 1:
                    return LLAST, LLAST, xlast, olast, 0
                q, k = c // PACKW, c % PACKW
                return 128, 128, xt[q], ot[q], k * R

            ps_of = {}
            src_of = {}  # chunk -> (out tile, free-offset) holding carries

            def emit_a(c):
                K, M, mov, dst, koff = chunk_geom(c)
                ps = psp.tile([128, 2 * 512], F32, tag="ps", name=f"ps{c}")
                ps_of[c] = ps
                for rb in range(RB):
                    nc.tensor.matmul(
                        ps[0:M, 512 * rb:512 * (rb + 1)],
                        wpk[0:K, 128 * c:128 * c + M],
                        mov[0:K, koff + 512 * rb:koff + 512 * (rb + 1)],
                        start=True, stop=(c == 0))

            def emit_finish(c):
                # MM_b (carry-in from previous chunk's drained out tile),
                # then drain chunk c: rb0 on DVE, rb1 on ScalarE.
                K, M, mov, dst, koff = chunk_geom(c)
                ps = ps_of[c]
                if c > 0:
                    wcol = 128 * ((NWB - 1) if c == NCH - 1 else c)
                    pt, poff = src_of[c - 1]
                    for rb in range(RB):
                        nc.tensor.matmul(
                            ps[0:M, 512 * rb:512 * (rb + 1)],
                            wpk[96:98, wcol:wcol + M],
                            pt[96:98, poff + 512 * rb:poff + 512 * (rb + 1)],
                            start=False, stop=True, tile_position=(96, 0))
                nc.vector.tensor_scalar_add(
                    dst[0:M, koff:koff + 512], ps[0:M, 0:512],
                    bias_sb[0:M, c:c + 1])
                nc.scalar.activation(
                    dst[0:M, koff + 512:koff + 1024], ps[0:M, 512:1024],
                    mybir.ActivationFunctionType.Identity,
                    bias=bias_sb[0:M, c:c + 1])
                src_of[c] = (dst, koff)
                if c == NCH - 1:
                    nc.sync.dma_start(olast_d[:], olast[0:LLAST, :])
                elif c % PACKW == PACKW - 1:
                    q = c // PACKW
                    eng = nc.sync if q % 2 == 0 else nc.scalar
                    eng.dma_start(opk_d[:, q, :], ot[q][:])

            # Pipelined emission: MM_a(c+1) lands in the PE FIFO before
            # MM_b(c) so the PE has independent work while MM_b waits on
            # the previous drain.
            emit_a(0)
            emit_finish(0)
            for c in range(1, NCH):
                emit_a(c)
                if c >= 2:
                    emit_finish(c - 1)
            emit_finish(NCH - 1)

    nc.compile()
    return nc


def _host_prep(weight, bias, decay_value):
    w0 = weight[0].astype(np.float64)
    w1 = weight[1].astype(np.float64)
    d0 = float(np.clip(np.float32(decay_value[0, 0]), 0.9, 1.0)) ** (1.0 / DC)
    d1 = float(np.clip(np.float32(decay_value[1, 0]), 0.9, 1.0)) ** (1.0 / DC)

    wpk = np.zeros((128, NWB * 128), dtype=np.float64)
    biasT = np.zeros((128, NCH), dtype=np.float32)
    for c in range(NCH):
        s0 = c * L
        Lc = L if c < NFULL else LLAST
        w = np.zeros((128, 128))
        ii = np.arange(Lc)
        jj = np.arange(Lc)
        msk = jj[None, :] >= ii[:, None]
        pw = np.where(msk, jj[None, :] - ii[:, None], 0)
        w[:Lc, :Lc] = np.where(
            msk,
            w0[s0 + ii][:, None] * d0 ** pw + w1[s0 + jj][None, :] * d1 ** pw,
            0.0)
        w[Lc, :Lc] = d0 ** (jj + 1)
        w[Lc + 1, :Lc] = w1[s0 + jj] * d1 ** (jj + 1)
        if c < NFULL:
            w[:Lc, Lc] = w0[s0 + ii] * d0 ** (Lc - 1 - ii)
            w[:Lc, Lc + 1] = d1 ** (Lc - 1 - ii)
            w[Lc, Lc] = d0 ** Lc
            w[Lc + 1, Lc + 1] = d1 ** Lc
        bcol = np.zeros(128, dtype=np.float32)
        bcol[:Lc] = bias[s0:s0 + Lc]
        if c < NFULL:
            w = w[np.ix_(PERM, PERM)]
            bcol = bcol[PERM]
        else:
            # last chunk: carry-response rows move to the extra block
            # at partitions 96..97 (uniform MM_b addressing)
            wpk[96:98, 128 * NCH:128 * NCH + Lc] = w[Lc:Lc + 2, :Lc]
            w[Lc:Lc + 2, :] = 0.0
        wpk[:, 128 * c:128 * (c + 1)] = w
        biasT[:, c] = bcol
    return wpk.astype(np.float16), biasT


# gather indices: IDX[p, c] = global s row for partition p of full chunk c
_IDX = np.zeros((128, NFULL), dtype=np.int64)
_VALID = np.ones(128, dtype=bool)
for _p in range(128):
    if SREL[_p] is None:
        _VALID[_p] = False
        continue
    for _c in range(NFULL):
        _IDX[_p, _c] = _c * L + SREL[_p]


def prep_in_maps(x, weight, bias, decay_value):
    wpk, biasT = _host_prep(weight, bias, decay_value)
    x2 = np.asarray(x, dtype=np.float32).reshape(B * E, S)
    in_maps = []
    for core in range(N_CORES):
        xc = x2[R * core:R * (core + 1), :].astype(np.float16)
        xT = np.ascontiguousarray(xc.T)              # [S, R]
        xpk = xT[_IDX.T.reshape(-1), :].reshape(NFULL, 128, R)
        xpk = np.ascontiguousarray(xpk.transpose(1, 0, 2)).reshape(
            128, NPACK, PACKW * R)
        xpk[96:98, :, :] = 0
        xlast = np.ascontiguousarray(xT[SLAST:, :])
        in_maps.append({"xpk": xpk, "xlast": xlast, "wpk": wpk,
                        "biasT": biasT})
    return in_maps


def unpack_out(res_c):
    """Reassemble one core's [R, S] fp32 output from packed results."""
    opk = res_c["opk"].reshape(128, NFULL, R)
    outT = np.empty((S, R), dtype=np.float16)
    outT[_IDX[_VALID].reshape(-1), :] = opk[_VALID].reshape(-1, R)
    outT[SLAST:, :] = res_c["olast"]
    return outT.T.astype(np.float32)


def kernel(x, weight, bias, decay_value, index=0, recurrent=0, **_):
    global _PROGRAM
    x = np.asarray(x, dtype=np.float32)
    weight = np.asarray(weight, dtype=np.float32)
    bias = np.asarray(bias, dtype=np.float32)
    decay_value = np.asarray(decay_value, dtype=np.float32)

    if _PROGRAM is None:
        _PROGRAM = _build_program()

    in_maps = prep_in_maps(x, weight, bias, decay_value)
    res = run_bass_kernel_spmd(_PROGRAM, in_maps,
                               core_ids=list(range(N_CORES)))
    out = np.empty((B * E, S), dtype=np.float32)
    for c in range(N_CORES):
        out[R * c:R * (c + 1), :] = unpack_out(res.results[c])
    return out.reshape(B, E, S)


# revision 10
# speedup vs baseline: 1.2564x; 1.2564x over previous
"""Trainium2 Bass kernel for CombinedRepeatCausalLinear (parallel forward).

Computes out[b,e,t] = sum_s x[b,e,s] * W[s,t] + bias[t] where
  W[s,t] = mask(t>=s) * (w0[s]*d0^(t-s) + w1[t]*d1^(t-s))
for S = 2048, x of shape (8, 1024, 2048) fp32.

W is a causally-masked rank-2 matrix, so x @ W is a 2-state linear
recurrence along t, evaluated as a chunked scan: 17 diagonal chunks of
L=126 columns. Each chunk is ONE [128x128]-stationary matmul per
512-wide r-block:
  stationary rows <- 126 causal-decay x-rows + 2 incoming-carry rows
  psum cols       <- 126 out columns + 2 outgoing-carry (A,B) values
The two r-blocks form independent serial lanes, each owned end-to-end
by one drain engine (lane 0: DVE tensor_copy carry + tensor_scalar
out-drain; lane 1: ScalarE activation for both), so the carry copy and
the out drain never queue behind each other's lane.

Layout/perf notes:
- The PE clock stays throttled (1.2 GHz) for kernels this short, so
  the design minimizes PE passes: exactly 2 per chunk.
- Compute-engine SBUF accesses must start at a 32-aligned partition,
  so carry rows sit at partitions 96..97 and x/out rows occupy
  [0..95, 98..127] (stationary host-permuted to match). Last chunk
  (32 rows) keeps carries at 32..33.
- fp16 end-to-end I/O: 4MB x in + 4MB out + 0.5MB W per core.
- x and out are PACKED 2 chunks per DMA instruction ([128, 2048] fp16
  tiles = 4KB contiguous per partition; per-queue DMA throughput is
  packet-size-bound), alternating between the two HWDGE queues
  (sync + scalar).
"""

import numpy as np

import concourse.bass as bass
import concourse.mybir as mybir
import concourse.tile as tile
from concourse import bacc
from concourse.bass_utils import run_bass_kernel_spmd

F32 = mybir.dt.float32
F16 = mybir.dt.float16

B = 8
E = 1024
S = 2048
DC = 1.0
N_CORES = 8
R = (B * E) // N_CORES      # rows per core = 1024
L = 126                     # chunk length (+2 carry rows = 128 partitions)
RB = 2                      # r-blocks of 512 (fp32-psum bank width)
PACKW = 2                   # chunks per packed DMA
NPACK = 8                   # packs of full chunks (16 full chunks)
NFULL = NPACK * PACKW       # 16
SLAST = NFULL * L           # 2016
LLAST = S - SLAST           # 32
NCH = NFULL + 1             # 17

# partition p -> s_rel within a full chunk (96..97 are carry slots)
SREL = list(range(96)) + [None, None] + list(range(96, L))
# logical row/col permutation applied to the [128,128] stationary block
PERM = list(range(96)) + [126, 127] + list(range(96, 126))

_PROGRAM = None


def _build_program():
    nc = bacc.Bacc("TRN2", target_bir_lowering=False, debug=False,
                   num_devices=N_CORES)

    xpk_d = nc.declare_dram_parameter("xpk", [128, NPACK, PACKW * R], F16,
                                      isOutput=False)
    xlast_d = nc.declare_dram_parameter("xlast", [LLAST, R], F16,
                                        isOutput=False)
    wpk_d = nc.declare_dram_parameter("wpk", [128, NCH * 128], F16,
                                      isOutput=False)
    biasT_d = nc.declare_dram_parameter("biasT", [128, NCH], F32,
                                        isOutput=False)
    opk_d = nc.declare_dram_parameter("opk", [128, NPACK, PACKW * R], F16,
                                      isOutput=True)
    olast_d = nc.declare_dram_parameter("olast", [LLAST, R], F16,
                                        isOutput=True)

    with tile.TileContext(nc) as tc:
        with (
            tc.tile_pool(name="cst", bufs=1) as cst,
            tc.tile_pool(name="xp", bufs=1) as xp,
            tc.tile_pool(name="op", bufs=1) as op,
            tc.tile_pool(name="ps", bufs=6, space="PSUM") as psp,
        ):
            # ---- constants: packed W + bias (scalar queue, first) ----
            wpk = cst.tile([128, NCH * 128], F16, tag="wpk")
            nc.scalar.dma_start(wpk[:], wpk_d[:])
            bias_sb = cst.tile([128, NCH], F32, tag="bias")
            nc.scalar.dma_start(bias_sb[:], biasT_d[:])

            # ---- x input: packs alternating across the two HWDGE queues ----
            xt = []
            for q in range(NPACK):
                t = xp.tile([128, PACKW * R], F16, tag=f"xq{q}",
                            name=f"xq{q}")
                eng = nc.sync if q % 2 == 0 else nc.scalar
                eng.dma_start(t[:], xpk_d[:, q, :])
                xt.append(t)
            xlast = xp.tile([LLAST + 2, R], F16, tag="xlast")
            nc.sync.dma_start(xlast[0:LLAST, :], xlast_d[:])

            ot = [op.tile([128, PACKW * R], F16, tag=f"oq{q}", name=f"oq{q}")
                  for q in range(NPACK)]
            olast = op.tile([128, R], F16, tag="olast")

            for c in range(NCH):
                last = (c == NCH - 1)
                q, k = c // PACKW, c % PACKW
                if last:
                    K, M = LLAST + 2, LLAST
                    mov, dst, koff = xlast, olast, 0
                else:
                    K, M = 128, 128
                    mov, dst, koff = xt[q], ot[q], k * R
                cpos = 96 if not last else LLAST
                for rb in range(RB):
                    fs = slice(koff + 512 * rb, koff + 512 * (rb + 1))
                    ps = psp.tile([128, 512], F32, tag="ps",
                                  name=f"ps{c}_{rb}")
                    nc.tensor.matmul(ps[0:M, :],
                                     wpk[0:K, 128 * c:128 * c + M],
                                     mov[0:K, fs], start=True, stop=True)
                    if not last:
                        # carry handoff to the next chunk's moving tile
                        if c + 1 == NCH - 1:
                            ndst = xlast
                            ncp = LLAST
                            nfs = slice(512 * rb, 512 * (rb + 1))
                        else:
                            nq, nk = (c + 1) // PACKW, (c + 1) % PACKW
                            ndst = xt[nq]
                            ncp = 96
                            nfs = slice(nk * R + 512 * rb,
                                        nk * R + 512 * (rb + 1))
                        if rb == 0:
                            nc.vector.tensor_copy(ndst[ncp:ncp + 2, nfs],
                                                  ps[96:98, :])
                        else:
                            nc.scalar.activation(
                                ndst[ncp:ncp + 2, nfs], ps[96:98, :],
                                mybir.ActivationFunctionType.Copy)
                    # out drain (with bias): lane 0 on DVE, lane 1 on ScalarE
                    if rb == 0:
                        nc.vector.tensor_scalar_add(
                            dst[0:M, fs], ps[0:M, :], bias_sb[0:M, c:c + 1])
                    else:
                        nc.scalar.activation(
                            dst[0:M, fs], ps[0:M, :],
                            mybir.ActivationFunctionType.Identity,
                            bias=bias_sb[0:M, c:c + 1])
                if last:
                    nc.sync.dma_start(olast_d[:], olast[0:LLAST, :])
                elif k == PACKW - 1:
                    eng = nc.sync if q % 2 == 0 else nc.scalar
                    eng.dma_start(opk_d[:, q, :], ot[q][:])

    nc.compile()
    return nc


def _host_prep(weight, bias, decay_value):
    w0 = weight[0].astype(np.float64)
    w1 = weight[1].astype(np.float64)
    d0 = float(np.clip(np.float32(decay_value[0, 0]), 0.9, 1.0)) ** (1.0 / DC)
    d1 = float(np.clip(np.float32(decay_value[1, 0]), 0.9, 1.0)) ** (1.0 / DC)

    wpk = np.zeros((128, NCH * 128), dtype=np.float64)
    biasT = np.zeros((128, NCH), dtype=np.float32)
    for c in range(NCH):
        s0 = c * L
        Lc = L if c < NFULL else LLAST
        w = np.zeros((128, 128))
        ii = np.arange(Lc)
        jj = np.arange(Lc)
        msk = jj[None, :] >= ii[:, None]
        pw = np.where(msk, jj[None, :] - ii[:, None], 0)
        w[:Lc, :Lc] = np.where(
            msk,
            w0[s0 + ii][:, None] * d0 ** pw + w1[s0 + jj][None, :] * d1 ** pw,
            0.0)
        w[Lc, :Lc] = d0 ** (jj + 1)
        w[Lc + 1, :Lc] = w1[s0 + jj] * d1 ** (jj + 1)
        if c < NFULL:
            w[:Lc, Lc] = w0[s0 + ii] * d0 ** (Lc - 1 - ii)
            w[:Lc, Lc + 1] = d1 ** (Lc - 1 - ii)
            w[Lc, Lc] = d0 ** Lc
            w[Lc + 1, Lc + 1] = d1 ** Lc
        bcol = np.zeros(128, dtype=np.float32)
        bcol[:Lc] = bias[s0:s0 + Lc]
        if c < NFULL:
            w = w[np.ix_(PERM, PERM)]
            bcol = bcol[PERM]
        wpk[:, 128 * c:128 * (c + 1)] = w
        biasT[:, c] = bcol
    return wpk.astype(np.float16), biasT


# gather indices: IDX[p, c] = global s row for partition p of full chunk c
_IDX = np.zeros((128, NFULL), dtype=np.int64)
_VALID = np.ones(128, dtype=bool)
for _p in range(128):
    if SREL[_p] is None:
        _VALID[_p] = False
        continue
    for _c in range(NFULL):
        _IDX[_p, _c] = _c * L + SREL[_p]


def prep_in_maps(x, weight, bias, decay_value):
    wpk, biasT = _host_prep(weight, bias, decay_value)
    x2 = np.asarray(x, dtype=np.float32).reshape(B * E, S)
    in_maps = []
    for core in range(N_CORES):
        xc = x2[R * core:R * (core + 1), :].astype(np.float16)
        xT = np.ascontiguousarray(xc.T)              # [S, R]
        xpk = xT[_IDX.T.reshape(-1), :].reshape(NFULL, 128, R)
        xpk = np.ascontiguousarray(xpk.transpose(1, 0, 2)).reshape(
            128, NPACK, PACKW * R)
        xpk[96:98, :, :] = 0
        xlast = np.ascontiguousarray(xT[SLAST:, :])
        in_maps.append({"xpk": xpk, "xlast": xlast, "wpk": wpk,
                        "biasT": biasT})
    return in_maps


def unpack_out(res_c):
    """Reassemble one core's [R, S] fp32 output from packed results."""
    opk = res_c["opk"].reshape(128, NFULL, R)
    outT = np.empty((S, R), dtype=np.float16)
    outT[_IDX[_VALID].reshape(-1), :] = opk[_VALID].reshape(-1, R)
    outT[SLAST:, :] = res_c["olast"]
    return outT.T.astype(np.float32)


def kernel(x, weight, bias, decay_value, index=0, recurrent=0, **_):
    global _PROGRAM
    x = np.asarray(x, dtype=np.float32)
    weight = np.asarray(weight, dtype=np.float32)
    bias = np.asarray(bias, dtype=np.float32)
    decay_value = np.asarray(decay_value, dtype=np.float32)

    if _PROGRAM is None:
        _PROGRAM = _build_program()

    in_maps = prep_in_maps(x, weight, bias, decay_value)
    res = run_bass_kernel_spmd(_PROGRAM, in_maps,
                               core_ids=list(range(N_CORES)))
    out = np.empty((B * E, S), dtype=np.float32)
    for c in range(N_CORES):
        out[R * c:R * (c + 1), :] = unpack_out(res.results[c])
    return out.reshape(B, E, S)


# revision 13
# speedup vs baseline: 1.2816x; 1.0201x over previous
"""Trainium2 Bass kernel for CombinedRepeatCausalLinear (parallel forward).

Computes out[b,e,t] = sum_s x[b,e,s] * W[s,t] + bias[t] where
  W[s,t] = mask(t>=s) * (w0[s]*d0^(t-s) + w1[t]*d1^(t-s))
for S = 2048, x of shape (8, 1024, 2048) fp32.

W is a causally-masked rank-2 matrix, so x @ W is a 2-state linear
recurrence along t, evaluated as a chunked scan: 17 diagonal chunks of
L=126 columns. Each chunk is ONE [128x128]-stationary matmul per
512-wide r-block:
  stationary rows <- 126 causal-decay x-rows + 2 incoming-carry rows
  psum cols       <- 126 out columns + 2 outgoing-carry (A,B) values
The two r-blocks form independent serial lanes, each owned end-to-end
by one drain engine (lane 0: DVE tensor_copy carry + tensor_scalar
out-drain; lane 1: ScalarE activation for both), so the carry copy and
the out drain never queue behind each other's lane.

Layout/perf notes:
- The PE clock stays throttled (1.2 GHz) for kernels this short, so
  the design minimizes PE passes: exactly 2 per chunk.
- Compute-engine SBUF accesses must start at a 32-aligned partition,
  so carry rows sit at partitions 96..97 and x/out rows occupy
  [0..95, 98..127] (stationary host-permuted to match). Last chunk
  (32 rows) keeps carries at 32..33.
- fp16 end-to-end I/O: 4MB x in + 4MB out + 0.5MB W per core.
- x and out are PACKED 2 chunks per DMA instruction ([128, 2048] fp16
  tiles = 4KB contiguous per partition; per-queue DMA throughput is
  packet-size-bound), alternating between the two HWDGE queues
  (sync + scalar).
"""

import numpy as np

import concourse.bass as bass
import concourse.mybir as mybir
import concourse.tile as tile
from concourse import bacc
from concourse.bass_utils import run_bass_kernel_spmd

F32 = mybir.dt.float32
F16 = mybir.dt.float16

B = 8
E = 1024
S = 2048
DC = 1.0
N_CORES = 8
R = (B * E) // N_CORES      # rows per core = 1024
L = 126                     # chunk length (+2 carry rows = 128 partitions)
RB = 2                      # r-blocks of 512 (fp32-psum bank width)
PACKW = 2                   # chunks per packed DMA
NPACK = 8                   # packs of full chunks (16 full chunks)
NFULL = NPACK * PACKW       # 16
SLAST = NFULL * L           # 2016
LLAST = S - SLAST           # 32
NCH = NFULL + 1             # 17

# partition p -> s_rel within a full chunk (96..97 are carry slots)
SREL = list(range(96)) + [None, None] + list(range(96, L))
# logical row/col permutation applied to the [128,128] stationary block
PERM = list(range(96)) + [126, 127] + list(range(96, 126))

_PROGRAM = None


def _build_program():
    nc = bacc.Bacc("TRN2", target_bir_lowering=False, debug=False,
                   num_devices=N_CORES)

    xpk_d = nc.declare_dram_parameter("xpk", [128, NPACK, PACKW * R], F16,
                                      isOutput=False)
    xlast_d = nc.declare_dram_parameter("xlast", [LLAST, R], F16,
                                        isOutput=False)
    wpk_d = nc.declare_dram_parameter("wpk", [128, NCH * 128], F16,
                                      isOutput=False)
    biasT_d = nc.declare_dram_parameter("biasT", [128, NCH], F32,
                                        isOutput=False)
    opk_d = nc.declare_dram_parameter("opk", [128, NPACK, PACKW * R], F16,
                                      isOutput=True)
    olast_d = nc.declare_dram_parameter("olast", [LLAST, R], F16,
                                        isOutput=True)

    with tile.TileContext(nc) as tc:
        with (
            tc.tile_pool(name="cst", bufs=1) as cst,
            tc.tile_pool(name="xp", bufs=1) as xp,
            tc.tile_pool(name="op", bufs=1) as op,
            tc.tile_pool(name="ps", bufs=6, space="PSUM") as psp,
        ):
            # ---- constants: W split so early chunks unblock fast; the
            # scalar HWDGE queue cold-starts ~3us later than sync, so
            # blocks 0..5 ride the sync queue first ----
            WSPLIT = 6 * 128
            wpk = cst.tile([128, NCH * 128], F16, tag="wpk")
            nc.sync.dma_start(wpk[:, 0:WSPLIT], wpk_d[:, 0:WSPLIT])
            nc.scalar.dma_start(wpk[:, WSPLIT:], wpk_d[:, WSPLIT:])
            bias_sb = cst.tile([128, NCH], F32, tag="bias")
            nc.scalar.dma_start(bias_sb[:], biasT_d[:])

            # ---- x input: packs alternating across the two HWDGE
            # queues; pack 0 is split per chunk so chunk 0 starts ASAP ----
            xt = []
            for q in range(NPACK):
                t = xp.tile([128, PACKW * R], F16, tag=f"xq{q}",
                            name=f"xq{q}")
                eng = nc.sync if q % 2 == 0 else nc.scalar
                if q == 0:
                    for k in range(PACKW):
                        eng.dma_start(t[:, k * R:(k + 1) * R],
                                      xpk_d[:, q, k * R:(k + 1) * R])
                else:
                    eng.dma_start(t[:], xpk_d[:, q, :])
                xt.append(t)
            xlast = xp.tile([LLAST + 2, R], F16, tag="xlast")
            nc.sync.dma_start(xlast[0:LLAST, :], xlast_d[:])

            ot = [op.tile([128, PACKW * R], F16, tag=f"oq{q}", name=f"oq{q}")
                  for q in range(NPACK)]
            olast = op.tile([128, R], F16, tag="olast")

            def geom(c):
                if c == NCH - 1:
                    return LLAST + 2, LLAST, xlast, olast, 0
                q, k = c // PACKW, c % PACKW
                return 128, 128, xt[q], ot[q], k * R

            # rb0's out-drain (DVE tensor_scalar) is deferred one chunk
            # so the chain-critical carry CAST never queues behind it
            pending_ts = []

            def flush_ts():
                for (dst, M, fs, ps, c) in pending_ts:
                    nc.vector.tensor_scalar_add(
                        dst[0:M, fs], ps[0:M, :], bias_sb[0:M, c:c + 1])
                pending_ts.clear()

            for c in range(NCH):
                last = (c == NCH - 1)
                K, M, mov, dst, koff = geom(c)
                for rb in range(RB):
                    fs = slice(koff + 512 * rb, koff + 512 * (rb + 1))
                    ps = psp.tile([128, 512], F32, tag="ps",
                                  name=f"ps{c}_{rb}")
                    nc.tensor.matmul(ps[0:M, :],
                                     wpk[0:K, 128 * c:128 * c + M],
                                     mov[0:K, fs], start=True, stop=True)
                    if not last:
                        # carry handoff to the next chunk's moving tile
                        if c + 1 == NCH - 1:
                            ndst, ncp = xlast, LLAST
                            nfs = slice(512 * rb, 512 * (rb + 1))
                        else:
                            nq, nk = (c + 1) // PACKW, (c + 1) % PACKW
                            ndst, ncp = xt[nq], 96
                            nfs = slice(nk * R + 512 * rb,
                                        nk * R + 512 * (rb + 1))
                        if rb == 0:
                            nc.vector.tensor_copy(ndst[ncp:ncp + 2, nfs],
                                                  ps[96:98, :])
                        else:
                            nc.scalar.activation(
                                ndst[ncp:ncp + 2, nfs], ps[96:98, :],
                                mybir.ActivationFunctionType.Copy)
                    if rb == 0:
                        # previous chunk's deferred DVE drain, then queue
                        # this chunk's
                        flush_ts()
                        pending_ts.append((dst, M, fs, ps, c))
                    else:
                        nc.scalar.activation(
                            dst[0:M, fs], ps[0:M, :],
                            mybir.ActivationFunctionType.Identity,
                            bias=bias_sb[0:M, c:c + 1])
                # per-chunk out DMA for the fully-drained previous chunk
                if 0 < c <= NFULL:
                    pc = c - 1
                    pq, pk = pc // PACKW, pc % PACKW
                    eng = nc.sync if pc % 2 == 0 else nc.scalar
                    eng.dma_start(opk_d[:, pq, pk * R:(pk + 1) * R],
                                  ot[pq][:, pk * R:(pk + 1) * R])
            flush_ts()
            nc.sync.dma_start(olast_d[:], olast[0:LLAST, :])

    nc.compile()
    return nc


def _host_prep(weight, bias, decay_value):
    w0 = weight[0].astype(np.float64)
    w1 = weight[1].astype(np.float64)
    d0 = float(np.clip(np.float32(decay_value[0, 0]), 0.9, 1.0)) ** (1.0 / DC)
    d1 = float(np.clip(np.float32(decay_value[1, 0]), 0.9, 1.0)) ** (1.0 / DC)

    wpk = np.zeros((128, NCH * 128), dtype=np.float64)
    biasT = np.zeros((128, NCH), dtype=np.float32)
    for c in range(NCH):
        s0 = c * L
        Lc = L if c < NFULL else LLAST
        w = np.zeros((128, 128))
        ii = np.arange(Lc)
        jj = np.arange(Lc)
        msk = jj[None, :] >= ii[:, None]
        pw = np.where(msk, jj[None, :] - ii[:, None], 0)
        w[:Lc, :Lc] = np.where(
            msk,
            w0[s0 + ii][:, None] * d0 ** pw + w1[s0 + jj][None, :] * d1 ** pw,
            0.0)
        w[Lc, :Lc] = d0 ** (jj + 1)
        w[Lc + 1, :Lc] = w1[s0 + jj] * d1 ** (jj + 1)
        if c < NFULL:
            w[:Lc, Lc] = w0[s0 + ii] * d0 ** (Lc - 1 - ii)
            w[:Lc, Lc + 1] = d1 ** (Lc - 1 - ii)
            w[Lc, Lc] = d0 ** Lc
            w[Lc + 1, Lc + 1] = d1 ** Lc
        bcol = np.zeros(128, dtype=np.float32)
        bcol[:Lc] = bias[s0:s0 + Lc]
        if c < NFULL:
            w = w[np.ix_(PERM, PERM)]
            bcol = bcol[PERM]
        wpk[:, 128 * c:128 * (c + 1)] = w
        biasT[:, c] = bcol
    return wpk.astype(np.float16), biasT


# gather indices: IDX[p, c] = global s row for partition p of full chunk c
_IDX = np.zeros((128, NFULL), dtype=np.int64)
_VALID = np.ones(128, dtype=bool)
for _p in range(128):
    if SREL[_p] is None:
        _VALID[_p] = False
        continue
    for _c in range(NFULL):
        _IDX[_p, _c] = _c * L + SREL[_p]


def prep_in_maps(x, weight, bias, decay_value):
    wpk, biasT = _host_prep(weight, bias, decay_value)
    x2 = np.asarray(x, dtype=np.float32).reshape(B * E, S)
    in_maps = []
    for core in range(N_CORES):
        xc = x2[R * core:R * (core + 1), :].astype(np.float16)
        xT = np.ascontiguousarray(xc.T)              # [S, R]
        xpk = xT[_IDX.T.reshape(-1), :].reshape(NFULL, 128, R)
        xpk = np.ascontiguousarray(xpk.transpose(1, 0, 2)).reshape(
            128, NPACK, PACKW * R)
        xpk[96:98, :, :] = 0
        xlast = np.ascontiguousarray(xT[SLAST:, :])
        in_maps.append({"xpk": xpk, "xlast": xlast, "wpk": wpk,
                        "biasT": biasT})
    return in_maps


def unpack_out(res_c):
    """Reassemble one core's [R, S] fp32 output from packed results."""
    opk = res_c["opk"].reshape(128, NFULL, R)
    outT = np.empty((S, R), dtype=np.float16)
    outT[_IDX[_VALID].reshape(-1), :] = opk[_VALID].reshape(-1, R)
    outT[SLAST:, :] = res_c["olast"]
    return outT.T.astype(np.float32)


def kernel(x, weight, bias, decay_value, index=0, recurrent=0, **_):
    global _PROGRAM
    x = np.asarray(x, dtype=np.float32)
    weight = np.asarray(weight, dtype=np.float32)
    bias = np.asarray(bias, dtype=np.float32)
    decay_value = np.asarray(decay_value, dtype=np.float32)

    if _PROGRAM is None:
        _PROGRAM = _build_program()

    in_maps = prep_in_maps(x, weight, bias, decay_value)
    res = run_bass_kernel_spmd(_PROGRAM, in_maps,
                               core_ids=list(range(N_CORES)))
    out = np.empty((B * E, S), dtype=np.float32)
    for c in range(N_CORES):
        out[R * c:R * (c + 1), :] = unpack_out(res.results[c])
    return out.reshape(B, E, S)
